# revision 11
# baseline (speedup 1.0000x reference)
"""Trainium2 Bass kernel for BondEncoding2D (Graphormer-style bond encoding).

Computes, for a 512x512 node-pair grid:
  phi_spd[h,i,j]  = spd_table[spatial_pos[i,j], h]
  phi_edge[h,i,j] = (sum_d edge_table[edge_input[i,j,d]] @ W[d])[h] / max(spatial_pos[i,j],1)

Sharding: rows of the grid across 8 NeuronCores (64 rows / 32768 pairs each);
tables and weights replicated (per the sharding hint).

Per-core strategy (everything on the TensorEngine):
  * Host precomputes M[d] = edge_table @ W[d] (bf16) and exact fp8 one-hot
    encodings of the indices (SBUF-resident, like the baseline's feature
    tensor): edge one-hot over c=(hop,bond) in 4 chunks of 128 rows, and a
    spd one-hot-64 with two pairs packed per 128-row column.
  * phi_edge: per group of 8 row-tiles, 32 matmuls (4 column-packed 32-wide
    tile_positions x 4 accumulated c-chunks x 2 bank-halves) compute
    edge_sum[h,pair] into a 2-bank PSUM tile [128, 1024]. Column packing runs
    the 4 narrow matmuls concurrently in the PE array.
  * phi_spd: per group of 2048 pairs, 2 column-packed matmuls
    (block-diagonal spd_table stationary [128,64]) -> PSUM [128, 512].
  * Epilogues alternate DVE/ACT: edge = tensor_tensor multiply by the
    replicated 1/max(spatial_pos,1); spd = plain copy; both -> bf16 staging.
  * Output DMAs are issued through GPSIMD's SWDGE (the Q7 cores are
    otherwise idle), keeping the ~1us/DMA descriptor-generation cost off
    the SP sequencer's critical path.
  * Outputs transfer in bf16; host casts to f32 and reassembles.
"""

import numpy as np
import ml_dtypes

import concourse.bass as bass
import concourse.bacc as bacc
import concourse.mybir as mybir
import concourse.tile as tile
from concourse.bass_utils import run_bass_kernel_spmd

N = 512          # atoms
D = 16           # max_dist
H = 32           # heads
NS = 64          # spatial values
NCORES = 8
RC = N // NCORES          # rows per core (64)
PC = RC * N               # pairs per core (32768)

TILES = 64                # edge row-tiles per core (one grid row each)
TP = 512                  # pairs per tile
NQ = 4                    # c-chunks (4 hops x 32 bonds each)
NEG = TILES // 8          # edge PE groups (8 tiles / 2 banks each) = 8
NSG = 16                  # spd groups (2048 pairs each)

BF16 = mybir.dt.bfloat16
F32 = mybir.dt.float32
FP8 = mybir.dt.float8e4

_cached = {}


def _build_nc(bench_reps=None, parts=("edge", "spd", "epi", "dma", "gpd", "ilv")):
    nc = bacc.Bacc(None, target_bir_lowering=False)

    onehot = nc.dram_tensor("onehot", [128, TILES * NQ * TP], FP8,
                            kind="ExternalInput")
    sphot = nc.dram_tensor("sphot", [128, PC // 2], FP8, kind="ExternalInput")
    mtab = nc.dram_tensor("mtab", [128, NQ * H], BF16, kind="ExternalInput")
    sblk = nc.dram_tensor("sblk", [128, 2 * H], BF16, kind="ExternalInput")
    rrep = nc.dram_tensor("rrep", [128, TILES * TP // 4], BF16,
                          kind="ExternalInput")
    oedge = nc.dram_tensor("oedge", [128, TILES * TP // 4], BF16,
                           kind="ExternalOutput")
    ospd = nc.dram_tensor("ospd", [128, PC // 4], BF16,
                          kind="ExternalOutput")

    mult = mybir.AluOpType.mult
    COPY = mybir.ActivationFunctionType.Copy

    with tile.TileContext(nc) as tc:
        with (
            tc.tile_pool(name="consts", bufs=1) as cpool,
            tc.tile_pool(name="pse", bufs=3, space="PSUM") as pepool,
            tc.tile_pool(name="pss", bufs=2, space="PSUM") as pspool,
            tc.tile_pool(name="ste", bufs=3) as stepool,
            tc.tile_pool(name="sts", bufs=3) as stspool,
        ):
            oh_t = cpool.tile([128, TILES * NQ * TP], FP8)
            nc.sync.dma_start(oh_t[:], onehot[:])
            sp_t = cpool.tile([128, PC // 2], FP8)
            nc.sync.dma_start(sp_t[:], sphot[:])
            mt_t = cpool.tile([128, NQ * H], BF16)
            nc.sync.dma_start(mt_t[:], mtab[:])
            sb_t = cpool.tile([128, 2 * H], BF16)
            nc.sync.dma_start(sb_t[:], sblk[:])
            rr_t = cpool.tile([128, TILES * TP // 4], BF16)
            nc.sync.dma_start(rr_t[:], rrep[:])

            import contextlib
            loop_cm = (
                tc.For_i(0, bench_reps, 1) if bench_reps
                else contextlib.nullcontext()
            )
            with loop_cm:
                # 8 superblocks; each: 1 edge group (8 tiles, 2 PSUM banks)
                # + 2 spd groups (4096 pairs, 2 PSUM banks)
                for g in [gg % NEG for gg in
                          range(2 * NEG if "u2" in parts else NEG)]:
                    # ---- edge: 2 bank-halves x (4 chunks x 4 col-tiles)
                    ps = pepool.tile([128, 2 * TP], F32, tag="ps")
                    if "edge" in parts:
                        if "ilv" in parts:
                            # q-outer: each strip runs two independent
                            # accumulation chains (bh=0/1) back to back
                            order = [(bh, q, ct) for q in range(NQ)
                                     for bh in range(2) for ct in range(4)]
                        else:
                            order = [(bh, q, ct) for bh in range(2)
                                     for q in range(NQ) for ct in range(4)]
                        for bh, q, ct in order:
                            t = 8 * g + 4 * bh + ct
                            nc.tensor.matmul(
                                ps[32 * ct:32 * ct + 32,
                                   TP * bh:TP * (bh + 1)],
                                mt_t[:, H * q:H * q + H],
                                oh_t[:, (t * NQ + q) * TP:
                                     (t * NQ + q + 1) * TP],
                                start=(q == 0), stop=(q == NQ - 1),
                                tile_position=(0, 32 * ct),
                            )
                    # ---- spd: 2 groups x 2 col-packed MMs [64, 512]
                    ps2s = []
                    for u in range(2 if "spd" in parts else 0):
                        sg = 2 * g + u
                        ps2 = pspool.tile([128, TP], F32, tag="ps2")
                        for half in range(2):
                            m = 2 * sg + half
                            nc.tensor.matmul(
                                ps2[64 * half:64 * half + 64, :],
                                sb_t[:, :],
                                sp_t[:, m * TP:(m + 1) * TP],
                                start=True, stop=True,
                                tile_position=(0, 64 * half),
                            )
                        ps2s.append(ps2)
                    # ---- epilogues (alternate DVE / ACT)
                    if "edge" in parts and "epi" in parts:
                        st = stepool.tile([128, 2 * TP], BF16, tag="st")
                        eng_e = nc.vector if g % 2 == 0 else nc.any
                        eng_e.tensor_tensor(
                            st[:], ps[:],
                            rr_t[:, 2 * TP * g:2 * TP * (g + 1)], mult,
                        )
                        if "dma" in parts:
                            eng_d = (nc.gpsimd if "gpd" in parts
                                     else nc.sync)
                            eng_d.dma_start(
                                oedge[:, 2 * TP * g:2 * TP * (g + 1)], st[:])
                    if "spd" in parts and "epi" in parts:
                        st2 = stspool.tile([128, 2 * TP], BF16, tag="st2")
                        for u in range(2):
                            if (g + u) % 2 == 0:
                                nc.scalar.activation(
                                    st2[:, TP * u:TP * (u + 1)], ps2s[u][:],
                                    COPY)
                            else:
                                nc.vector.tensor_scalar_mul(
                                    st2[:, TP * u:TP * (u + 1)], ps2s[u][:],
                                    1.0)
                        if "dma" in parts:
                            eng_d = (nc.gpsimd if "gpd" in parts
                                     else nc.sync)
                            eng_d.dma_start(
                                ospd[:, 2 * TP * g:2 * TP * (g + 1)], st2[:])
    nc.compile()
    return nc


def _host_prep(spatial_pos, edge_input, max_dist, spd_table, edge_table,
               edge_dis_weight):
    """Build per-core input maps (all numpy)."""
    md = int(max_dist)
    assert md == D
    W = edge_dis_weight.reshape(-1, H, H)[:md].astype(np.float64)
    M = (edge_table.astype(np.float64) @ W)        # (16, 32, 32)

    # mtab[32*dd + b, 32*q + h] = M[4q + dd, b, h]
    mtab = np.zeros((128, NQ * H), np.float64)
    for q in range(NQ):
        for dd in range(4):
            mtab[32 * dd:32 * dd + 32, H * q:H * q + H] = M[4 * q + dd]
    mtab = mtab.astype(ml_dtypes.bfloat16)

    # sblk: block-diagonal spd_table for the 2-pair-packed one-hot-64
    sblk = np.zeros((128, 2 * H), np.float32)
    sblk[:NS, :H] = spd_table
    sblk[NS:, H:] = spd_table
    sblk = sblk.astype(ml_dtypes.bfloat16)

    ONE_FP8 = np.float32(1.0).astype(ml_dtypes.float8_e4m3).view(np.uint8)

    in_maps = []
    for c in range(NCORES):
        rows = slice(RC * c, RC * (c + 1))
        e = edge_input[rows]                       # (64, 512, 16) int32
        # edge one-hot: row 32*(d%4) + e, col (t*4 + d//4)*512 + j
        t, j, d = np.meshgrid(np.arange(RC), np.arange(N), np.arange(D),
                              indexing="ij")
        oh = np.zeros((128, TILES * NQ * TP), np.uint8)
        oh[(32 * (d % 4) + e).ravel(),
           ((t * NQ + d // 4) * TP + j).ravel()] = ONE_FP8
        oh = oh.view(ml_dtypes.float8_e4m3)

        sp = spatial_pos[rows].reshape(PC).astype(np.int64)
        # spd one-hot-64: row 64*(pair%2) + sp[pair], col pair//2
        sph = np.zeros((128, PC // 2), np.uint8)
        pr = np.arange(PC)
        sph[64 * (pr % 2) + sp, pr // 2] = ONE_FP8
        sph = sph.view(ml_dtypes.float8_e4m3)

        r = (1.0 / np.maximum(sp, 1)).astype(np.float32)
        # rrep[32*ct + h, 1024*g + 512*bh + j] = r[(8g + 4bh + ct)*512 + j]
        rr = r.reshape(NEG, 2, 4, TP).transpose(2, 0, 1, 3)  # [ct, g, bh, j]
        rr = rr.reshape(4, TILES * TP // 4)
        rrep = np.repeat(rr, 32, axis=0).astype(ml_dtypes.bfloat16)

        in_maps.append({
            "onehot": oh, "sphot": sph, "mtab": mtab, "sblk": sblk,
            "rrep": rrep,
        })
    return in_maps


def _host_assemble(results):
    phi_spd = np.empty((H, N, N), np.float32)
    phi_edge = np.empty((H, N, N), np.float32)
    for c in range(NCORES):
        rs = slice(RC * c, RC * (c + 1))
        # oedge[32*ct + h, 1024*g + 512*bh + j] -> phi_edge[h, 8g+4bh+ct, j]
        b = np.asarray(results[c]["oedge"]).astype(np.float32)
        b = b.reshape(4, H, NEG, 2, TP).transpose(1, 2, 3, 0, 4)
        phi_edge[:, rs, :] = b.reshape(H, RC, N)
        # ospd[64*half + 32*parity + h, 512*sg + j]
        #   -> pair = 2048*sg + 1024*half + 2*j + parity
        a = np.asarray(results[c]["ospd"]).astype(np.float32)
        a = a.reshape(2, 2, H, NSG, TP).transpose(2, 3, 0, 4, 1)
        phi_spd[:, rs, :] = a.reshape(H, RC, N)
    return phi_spd, phi_edge


def kernel(spatial_pos, edge_input, max_dist, spd_table, edge_table,
           edge_dis_weight, _trace=False):
    spatial_pos = np.asarray(spatial_pos)
    edge_input = np.asarray(edge_input)
    spd_table = np.asarray(spd_table, dtype=np.float32)
    edge_table = np.asarray(edge_table, dtype=np.float32)
    edge_dis_weight = np.asarray(edge_dis_weight, dtype=np.float32)

    if "nc" not in _cached:
        _cached["nc"] = _build_nc()
    nc = _cached["nc"]

    in_maps = _host_prep(spatial_pos, edge_input, max_dist, spd_table,
                         edge_table, edge_dis_weight)
    res = run_bass_kernel_spmd(
        nc, in_maps, core_ids=list(range(NCORES)), trace=bool(_trace)
    )
    out = _host_assemble(res.results)
    if _trace:
        return out, res
    return out


# revision 12
# speedup vs baseline: 1.0640x; 1.0640x over previous
"""Trainium2 Bass kernel for BondEncoding2D (Graphormer-style bond encoding).

Computes, for a 512x512 node-pair grid:
  phi_spd[h,i,j]  = spd_table[spatial_pos[i,j], h]
  phi_edge[h,i,j] = (sum_d edge_table[edge_input[i,j,d]] @ W[d])[h] / max(spatial_pos[i,j],1)

Sharding: rows of the grid across 8 NeuronCores (64 rows / 32768 pairs each);
tables and weights replicated (per the sharding hint).

Per-core strategy (everything on the TensorEngine):
  * Host precomputes M[d] = edge_table @ W[d] (bf16) and exact fp8 one-hot
    encodings of the indices (SBUF-resident, like the baseline's feature
    tensor): edge one-hot over c=(hop,bond) in 4 chunks of 128 rows, and a
    spd one-hot-64 with two pairs packed per 128-row column.
  * phi_edge: per group of 8 row-tiles, 32 matmuls (4 column-packed 32-wide
    tile_positions x 4 accumulated c-chunks x 2 bank-halves) compute
    edge_sum[h,pair] into a 2-bank PSUM tile [128, 1024]. Column packing runs
    the 4 narrow matmuls concurrently in the PE array.
  * phi_spd: per group of 2048 pairs, 2 column-packed matmuls
    (block-diagonal spd_table stationary [128,64]) -> PSUM [128, 512].
  * Epilogues alternate DVE/ACT: edge = tensor_tensor multiply by the
    replicated 1/max(spatial_pos,1); spd = plain copy; both -> bf16 staging.
  * Output DMAs are issued through GPSIMD's SWDGE (the Q7 cores are
    otherwise idle), keeping the ~1us/DMA descriptor-generation cost off
    the SP sequencer's critical path.
  * Outputs transfer in bf16; host casts to f32 and reassembles.
"""

import numpy as np
import ml_dtypes

import concourse.bass as bass
import concourse.bacc as bacc
import concourse.mybir as mybir
import concourse.tile as tile
from concourse.bass_utils import run_bass_kernel_spmd

N = 512          # atoms
D = 16           # max_dist
H = 32           # heads
NS = 64          # spatial values
NCORES = 8
RC = N // NCORES          # rows per core (64)
PC = RC * N               # pairs per core (32768)

TILES = 64                # edge row-tiles per core (one grid row each)
TP = 512                  # pairs per tile
NQ = 4                    # c-chunks (4 hops x 32 bonds each)
NEG = TILES // 8          # edge PE groups (8 tiles / 2 banks each) = 8
NSG = 16                  # spd groups (2048 pairs each)

BF16 = mybir.dt.bfloat16
F32 = mybir.dt.float32
FP8 = mybir.dt.float8e4

_cached = {}


def _build_nc(bench_reps=None, parts=("edge", "spd", "epi", "dma", "gpd", "ilv")):
    nc = bacc.Bacc(None, target_bir_lowering=False)

    onehot = nc.dram_tensor("onehot", [128, TILES * NQ * TP], FP8,
                            kind="ExternalInput")
    sphot = nc.dram_tensor("sphot", [128, PC // 2], FP8, kind="ExternalInput")
    mtab = nc.dram_tensor("mtab", [128, NQ * H], BF16, kind="ExternalInput")
    sblk = nc.dram_tensor("sblk", [128, 2 * H], BF16, kind="ExternalInput")
    rrep = nc.dram_tensor("rrep", [128, TILES * TP // 4], BF16,
                          kind="ExternalInput")
    oedge = nc.dram_tensor("oedge", [128, TILES * TP // 4], BF16,
                           kind="ExternalOutput")
    ospd = nc.dram_tensor("ospd", [128, PC // 4], BF16,
                          kind="ExternalOutput")

    mult = mybir.AluOpType.mult
    COPY = mybir.ActivationFunctionType.Copy

    with tile.TileContext(nc) as tc:
        with (
            tc.tile_pool(name="consts", bufs=1) as cpool,
            tc.tile_pool(name="pse", bufs=3, space="PSUM") as pepool,
            tc.tile_pool(name="pss", bufs=2, space="PSUM") as pspool,
            tc.tile_pool(name="ste", bufs=3) as stepool,
            tc.tile_pool(name="sts", bufs=3) as stspool,
        ):
            oh_t = cpool.tile([128, TILES * NQ * TP], FP8)
            nc.sync.dma_start(oh_t[:], onehot[:])
            sp_t = cpool.tile([128, PC // 2], FP8)
            nc.sync.dma_start(sp_t[:], sphot[:])
            mt_t = cpool.tile([128, NQ * H], BF16)
            nc.sync.dma_start(mt_t[:], mtab[:])
            sb_t = cpool.tile([128, 2 * H], BF16)
            nc.sync.dma_start(sb_t[:], sblk[:])
            rr_t = cpool.tile([128, TILES * TP // 4], BF16)
            nc.sync.dma_start(rr_t[:], rrep[:])

            import contextlib
            loop_cm = (
                tc.For_i(0, bench_reps, 1) if bench_reps
                else contextlib.nullcontext()
            )
            with loop_cm:
                # 8 superblocks; each: 1 edge group (8 tiles, 2 PSUM banks)
                # + 2 spd groups (4096 pairs, 2 PSUM banks)
                if "ilv2" in parts:
                    # group-PAIR interleave: each PE strip cycles 4
                    # independent accumulation chains (2 groups x 2 halves)
                    for gp in range(NEG // 2):
                        pss_ = [pepool.tile([128, 2 * TP], F32, tag="ps")
                                for _ in range(2)]
                        for q in range(NQ):
                            for gs_ in range(2):
                                for bh in range(2):
                                    for ct in range(4):
                                        g = 2 * gp + gs_
                                        t = 8 * g + 4 * bh + ct
                                        nc.tensor.matmul(
                                            pss_[gs_][32 * ct:32 * ct + 32,
                                                      TP * bh:TP * (bh + 1)],
                                            mt_t[:, H * q:H * q + H],
                                            oh_t[:, (t * NQ + q) * TP:
                                                 (t * NQ + q + 1) * TP],
                                            start=(q == 0),
                                            stop=(q == NQ - 1),
                                            tile_position=(0, 32 * ct),
                                        )
                        for gs_ in range(2):
                            g = 2 * gp + gs_
                            ps2s = []
                            for u in range(2):
                                sg = 2 * g + u
                                ps2 = pspool.tile([128, TP], F32, tag="ps2")
                                for half in range(2):
                                    m = 2 * sg + half
                                    nc.tensor.matmul(
                                        ps2[64 * half:64 * half + 64, :],
                                        sb_t[:, :],
                                        sp_t[:, m * TP:(m + 1) * TP],
                                        start=True, stop=True,
                                        tile_position=(0, 64 * half),
                                    )
                                ps2s.append(ps2)
                            st = stepool.tile([128, 2 * TP], BF16, tag="st")
                            eng_e = nc.vector if g % 2 == 0 else nc.any
                            eng_e.tensor_tensor(
                                st[:], pss_[gs_][:],
                                rr_t[:, 2 * TP * g:2 * TP * (g + 1)], mult)
                            nc.gpsimd.dma_start(
                                oedge[:, 2 * TP * g:2 * TP * (g + 1)], st[:])
                            st2 = stspool.tile([128, 2 * TP], BF16, tag="st2")
                            for u in range(2):
                                if (g + u) % 2 == 0:
                                    nc.scalar.activation(
                                        st2[:, TP * u:TP * (u + 1)],
                                        ps2s[u][:], COPY)
                                else:
                                    nc.vector.tensor_scalar_mul(
                                        st2[:, TP * u:TP * (u + 1)],
                                        ps2s[u][:], 1.0)
                            nc.gpsimd.dma_start(
                                ospd[:, 2 * TP * g:2 * TP * (g + 1)], st2[:])
                    _skip_main = True
                else:
                    _skip_main = False
                for g in ([] if "ilv2" in parts else
                          [gg % NEG for gg in
                           range(2 * NEG if "u2" in parts else NEG)]):
                    # ---- edge: 2 bank-halves x (4 chunks x 4 col-tiles)
                    ps = pepool.tile([128, 2 * TP], F32, tag="ps")
                    if "edge" in parts:
                        if "ilv" in parts:
                            # q-outer: each strip runs two independent
                            # accumulation chains (bh=0/1) back to back
                            order = [(bh, q, ct) for q in range(NQ)
                                     for bh in range(2) for ct in range(4)]
                        else:
                            order = [(bh, q, ct) for bh in range(2)
                                     for q in range(NQ) for ct in range(4)]
                        for bh, q, ct in order:
                            t = 8 * g + 4 * bh + ct
                            nc.tensor.matmul(
                                ps[32 * ct:32 * ct + 32,
                                   TP * bh:TP * (bh + 1)],
                                mt_t[:, H * q:H * q + H],
                                oh_t[:, (t * NQ + q) * TP:
                                     (t * NQ + q + 1) * TP],
                                start=(q == 0), stop=(q == NQ - 1),
                                tile_position=(0, 32 * ct),
                            )
                    # ---- spd: 2 groups x 2 col-packed MMs [64, 512]
                    ps2s = []
                    for u in range(2 if "spd" in parts else 0):
                        sg = 2 * g + u
                        ps2 = pspool.tile([128, TP], F32, tag="ps2")
                        for half in range(2):
                            m = 2 * sg + half
                            nc.tensor.matmul(
                                ps2[64 * half:64 * half + 64, :],
                                sb_t[:, :],
                                sp_t[:, m * TP:(m + 1) * TP],
                                start=True, stop=True,
                                tile_position=(0, 64 * half),
                            )
                        ps2s.append(ps2)
                    # ---- epilogues (alternate DVE / ACT)
                    if "edge" in parts and "epi" in parts:
                        st = stepool.tile([128, 2 * TP], BF16, tag="st")
                        eng_e = nc.vector if g % 2 == 0 else nc.any
                        eng_e.tensor_tensor(
                            st[:], ps[:],
                            rr_t[:, 2 * TP * g:2 * TP * (g + 1)], mult,
                        )
                        if "dma" in parts:
                            eng_d = (nc.gpsimd if "gpd" in parts
                                     else nc.sync)
                            eng_d.dma_start(
                                oedge[:, 2 * TP * g:2 * TP * (g + 1)], st[:])
                    if "spd" in parts and "epi" in parts:
                        st2 = stspool.tile([128, 2 * TP], BF16, tag="st2")
                        for u in range(2):
                            if (g + u) % 2 == 0:
                                nc.scalar.activation(
                                    st2[:, TP * u:TP * (u + 1)], ps2s[u][:],
                                    COPY)
                            else:
                                nc.vector.tensor_scalar_mul(
                                    st2[:, TP * u:TP * (u + 1)], ps2s[u][:],
                                    1.0)
                        if "dma" in parts:
                            eng_d = (nc.gpsimd if "gpd" in parts
                                     else nc.sync)
                            eng_d.dma_start(
                                ospd[:, 2 * TP * g:2 * TP * (g + 1)], st2[:])
    nc.compile()
    return nc


def _host_prep(spatial_pos, edge_input, max_dist, spd_table, edge_table,
               edge_dis_weight):
    """Build per-core input maps (all numpy)."""
    md = int(max_dist)
    assert md == D
    W = edge_dis_weight.reshape(-1, H, H)[:md].astype(np.float64)
    M = (edge_table.astype(np.float64) @ W)        # (16, 32, 32)

    # mtab[32*dd + b, 32*q + h] = M[4q + dd, b, h]
    mtab = np.zeros((128, NQ * H), np.float64)
    for q in range(NQ):
        for dd in range(4):
            mtab[32 * dd:32 * dd + 32, H * q:H * q + H] = M[4 * q + dd]
    mtab = mtab.astype(ml_dtypes.bfloat16)

    # sblk: block-diagonal spd_table for the 2-pair-packed one-hot-64
    sblk = np.zeros((128, 2 * H), np.float32)
    sblk[:NS, :H] = spd_table
    sblk[NS:, H:] = spd_table
    sblk = sblk.astype(ml_dtypes.bfloat16)

    ONE_FP8 = np.float32(1.0).astype(ml_dtypes.float8_e4m3).view(np.uint8)

    in_maps = []
    for c in range(NCORES):
        rows = slice(RC * c, RC * (c + 1))
        e = edge_input[rows]                       # (64, 512, 16) int32
        # edge one-hot: row 32*(d%4) + e, col (t*4 + d//4)*512 + j
        t, j, d = np.meshgrid(np.arange(RC), np.arange(N), np.arange(D),
                              indexing="ij")
        oh = np.zeros((128, TILES * NQ * TP), np.uint8)
        oh[(32 * (d % 4) + e).ravel(),
           ((t * NQ + d // 4) * TP + j).ravel()] = ONE_FP8
        oh = oh.view(ml_dtypes.float8_e4m3)

        sp = spatial_pos[rows].reshape(PC).astype(np.int64)
        # spd one-hot-64: row 64*(pair%2) + sp[pair], col pair//2
        sph = np.zeros((128, PC // 2), np.uint8)
        pr = np.arange(PC)
        sph[64 * (pr % 2) + sp, pr // 2] = ONE_FP8
        sph = sph.view(ml_dtypes.float8_e4m3)

        r = (1.0 / np.maximum(sp, 1)).astype(np.float32)
        # rrep[32*ct + h, 1024*g + 512*bh + j] = r[(8g + 4bh + ct)*512 + j]
        rr = r.reshape(NEG, 2, 4, TP).transpose(2, 0, 1, 3)  # [ct, g, bh, j]
        rr = rr.reshape(4, TILES * TP // 4)
        rrep = np.repeat(rr, 32, axis=0).astype(ml_dtypes.bfloat16)

        in_maps.append({
            "onehot": oh, "sphot": sph, "mtab": mtab, "sblk": sblk,
            "rrep": rrep,
        })
    return in_maps


def _host_assemble(results):
    phi_spd = np.empty((H, N, N), np.float32)
    phi_edge = np.empty((H, N, N), np.float32)
    for c in range(NCORES):
        rs = slice(RC * c, RC * (c + 1))
        # oedge[32*ct + h, 1024*g + 512*bh + j] -> phi_edge[h, 8g+4bh+ct, j]
        b = np.asarray(results[c]["oedge"]).astype(np.float32)
        b = b.reshape(4, H, NEG, 2, TP).transpose(1, 2, 3, 0, 4)
        phi_edge[:, rs, :] = b.reshape(H, RC, N)
        # ospd[64*half + 32*parity + h, 512*sg + j]
        #   -> pair = 2048*sg + 1024*half + 2*j + parity
        a = np.asarray(results[c]["ospd"]).astype(np.float32)
        a = a.reshape(2, 2, H, NSG, TP).transpose(2, 3, 0, 4, 1)
        phi_spd[:, rs, :] = a.reshape(H, RC, N)
    return phi_spd, phi_edge


def kernel(spatial_pos, edge_input, max_dist, spd_table, edge_table,
           edge_dis_weight, _trace=False):
    spatial_pos = np.asarray(spatial_pos)
    edge_input = np.asarray(edge_input)
    spd_table = np.asarray(spd_table, dtype=np.float32)
    edge_table = np.asarray(edge_table, dtype=np.float32)
    edge_dis_weight = np.asarray(edge_dis_weight, dtype=np.float32)

    if "nc" not in _cached:
        _cached["nc"] = _build_nc()
    nc = _cached["nc"]

    in_maps = _host_prep(spatial_pos, edge_input, max_dist, spd_table,
                         edge_table, edge_dis_weight)
    res = run_bass_kernel_spmd(
        nc, in_maps, core_ids=list(range(NCORES)), trace=bool(_trace)
    )
    out = _host_assemble(res.results)
    if _trace:
        return out, res
    return out
